# revision 18
# baseline (speedup 1.0000x reference)
"""Trainium2 Bass kernel for nn_DNATransformer_1073741824689.

6-layer GPT-style transformer (B=2, S=1024, D=1024, H=16, HD=64, V=4),
sequence-parallel across 8 NeuronCores:

- Core i owns query chunk i (128 tokens) of BOTH batches (256 tokens/core).
- Activations kept feature-major ([feature, token]) in SBUF; LayerNorm stats
  computed with PE ones-matmuls; LN gain/bias and biases folded into weights
  host-side.
- Per layer: QKV GEMM -> AllGather K^T and V (bf16, ring on separate silicon)
  -> causal attention (logits^T = K Q^T per key-block; exp on ScalarE with a
  per-partition bias that masks non-causal blocks; softmax denominator via a
  ones-column appended to V; diagonal block handled in a separate static step
  with a triangular multiplicative mask) -> out-proj GEMM (+residual) -> MLP
  with Silu (+residual).
- All matmuls in bf16 with fp32 PSUM accumulation; residual stream fp32.
"""

import numpy as np
import ml_dtypes

import concourse.bass as bass
import concourse.bacc as bacc
import concourse.tile as tile
import concourse.mybir as mybir
from concourse.bass_utils import run_bass_kernel_spmd

R = 8          # cores
B = 2          # batch
S = 1024       # sequence
D = 1024       # model dim
H = 16         # heads
HD = 64        # head dim
L = 6          # layers
V = 4          # vocab
CH = 128       # tokens per (core, batch) chunk
T = B * CH     # tokens per core = 256
NF = D // 128  # feature tiles = 8
MASK_NEG = -50.0

bf16 = mybir.dt.bfloat16
f32 = mybir.dt.float32
AF = mybir.ActivationFunctionType
ALU = mybir.AluOpType

_BUILT = None  # cached (nc,) — compile once per process


def _build(n_layers=L, do_attn=True, do_dense=True, do_mlp=True, attn_stage=4):
    nc = bacc.Bacc("TRN2", target_bir_lowering=False, debug=False, num_devices=R)

    # ---------------- DRAM I/O ----------------
    tok_d = nc.dram_tensor("tok", [V, D], f32, kind="ExternalInput")
    oh_d = nc.dram_tensor("onehot", [V, T], f32, kind="ExternalInput")
    posT_d = nc.dram_tensor("posT", [128, NF, T], f32, kind="ExternalInput")
    mb_d = nc.dram_tensor("maskbias", [128, 8], f32, kind="ExternalInput")
    tri_d = nc.dram_tensor("trimask", [128, 128], bf16, kind="ExternalInput")
    id_d = nc.dram_tensor("ident", [128, 128], bf16, kind="ExternalInput")
    onb_d = nc.dram_tensor("ones_b", [128, 1], bf16, kind="ExternalInput")
    onf_d = nc.dram_tensor("ones_f", [1, 128], f32, kind="ExternalInput")
    headw_d = nc.dram_tensor("headw", [D, V], bf16, kind="ExternalInput")
    headb_d = nc.dram_tensor("headb", [V, 1], f32, kind="ExternalInput")
    qkvw_d, qkvb_d, outw_d, outb_d = [], [], [], []
    fc1w_d, fc1b_d, fc2w_d, fc2b_d = [], [], [], []
    for l in range(L):
        qkvw_d.append(nc.dram_tensor(f"qkvw{l}", [D, 3 * D], bf16, kind="ExternalInput"))
        qkvb_d.append(nc.dram_tensor(f"qkvb{l}", [128, 3 * NF], f32, kind="ExternalInput"))
        outw_d.append(nc.dram_tensor(f"outw{l}", [D, D], bf16, kind="ExternalInput"))
        outb_d.append(nc.dram_tensor(f"outb{l}", [128, NF], f32, kind="ExternalInput"))
        fc1w_d.append(nc.dram_tensor(f"fc1w{l}", [D, 4 * D], bf16, kind="ExternalInput"))
        fc1b_d.append(nc.dram_tensor(f"fc1b{l}", [128, 4 * NF], f32, kind="ExternalInput"))
        fc2w_d.append(nc.dram_tensor(f"fc2w{l}", [4 * D, D], bf16, kind="ExternalInput"))
        fc2b_d.append(nc.dram_tensor(f"fc2b{l}", [128, NF], f32, kind="ExternalInput"))
    out_d = nc.dram_tensor("out_logits", [V, T], f32, kind="ExternalOutput")

    # collective bounce buffers (DRAM)
    kin_d = nc.dram_tensor("cc_kin", [64, H * T], bf16, kind="Internal")
    kout_d = nc.dram_tensor("cc_kout", [R * 64, H * T], bf16, kind="Internal",
                            addr_space="Shared")
    vin_d = nc.dram_tensor("cc_vin", [128, B * H * 65], bf16, kind="Internal")
    vout_d = nc.dram_tensor("cc_vout", [R * 128, B * H * 65], bf16, kind="Internal",
                            addr_space="Shared")

    rg = [list(range(R))]

    with tile.TileContext(nc) as tc:
        with (
            tc.tile_pool(name="persist", bufs=1) as pp,
            tc.tile_pool(name="weights", bufs=3) as wp,
            tc.tile_pool(name="wfc2", bufs=2) as wp2,
            tc.tile_pool(name="bias", bufs=2) as bp,
            tc.tile_pool(name="tmp", bufs=3) as tp,
            tc.tile_pool(name="kcache", bufs=1) as kcp,
            tc.tile_pool(name="pg", bufs=2, space="PSUM") as pgp,
        ):
            # ---------------- persistent SBUF ----------------
            tok_sb = pp.tile([V, D], f32)
            oh_sb = pp.tile([V, T], f32)
            mb_sb = pp.tile([128, 8], f32)
            tri_sb = pp.tile([128, 128], bf16)
            id_sb = pp.tile([128, 128], bf16)
            onb_sb = pp.tile([128, 1], bf16)
            onf_sb = pp.tile([1, 128], f32)
            x_sb = pp.tile([128, NF, T], f32)       # residual stream
            n_sb = pp.tile([128, NF, T], bf16)      # LN output
            xb_sb = pp.tile([128, NF, T], bf16)     # bf16 cast of x
            qT_sb = pp.tile([128, NF, T], bf16)
            # kT staging aliases xb (disjoint lifetimes: xb lives only inside
            # emit_ln; kT from QKV evac until the kin split-DMAs complete)
            kT_sb = xb_sb
            vT_sb = pp.tile([128, NF, T], bf16)
            vpre_sb = pp.tile([128, B, H, 65], bf16)   # local V nat + ones col
            qh_sb = pp.tile([64, H, T], bf16)          # Q per head at base 0
            khloc_sb = pp.tile([64, H, T], bf16)       # local K per head at base 0
            V_sb = pp.tile([128, B, 8, H, 65], bf16)   # gathered V nat + ones
            av_sb = pp.tile([128, B, H, HD], bf16)
            avT_sb = pp.tile([128, NF, T], bf16)
            a_sb = pp.tile([128, 4 * NF, T], bf16)     # MLP hidden
            headw_sb = pp.tile([128, NF, V], bf16)
            headb_sb = pp.tile([V, 1], f32)
            out_sb = pp.tile([V, T], f32)

            nc.sync.dma_start(tok_sb[:], tok_d[:])
            nc.sync.dma_start(oh_sb[:], oh_d[:])
            nc.sync.dma_start(mb_sb[:], mb_d[:])
            nc.sync.dma_start(tri_sb[:], tri_d[:])
            nc.sync.dma_start(id_sb[:], id_d[:])
            nc.sync.dma_start(onb_sb[:], onb_d[:])
            nc.sync.dma_start(onf_sb[:], onf_d[:])
            nc.sync.dma_start(headw_sb[:], headw_d.ap().rearrange("(f p) v -> p f v", p=128))
            nc.sync.dma_start(headb_sb[:], headb_d[:])
            nc.vector.memset(vpre_sb[:, :, :, 64:65], 1.0)

            # ---------------- embedding ----------------
            for f in range(NF):
                pos_t = tp.tile([128, T], f32, tag="lntmp", name=f"pos{f}")
                nc.sync.dma_start(pos_t[:], posT_d[:, f, :])
                ps = pgp.tile([128, T], f32, tag="pg")
                nc.tensor.matmul(ps[:], tok_sb[:, f * 128:(f + 1) * 128],
                                 oh_sb[:], start=True, stop=True)
                nc.vector.tensor_add(x_sb[:, f, :], ps[:], pos_t[:])

            # ---------------- layer norm helper ----------------
            def emit_ln(sp):
                """x_sb (fp32) -> n_sb (bf16), normalized per token (partition-dim stats)."""
                for f in range(NF):
                    nc.vector.tensor_copy(xb_sb[:, f, :], x_sb[:, f, :])
                ps_s = sp.tile([1, T], f32, tag="psts")
                ps_q = sp.tile([1, T], f32, tag="pstq")
                for f in range(NF):
                    nc.tensor.matmul(ps_s[:], onb_sb[:], xb_sb[:, f, :],
                                     start=(f == 0), stop=(f == NF - 1))
                for f in range(NF):
                    sq = tp.tile([128, T], bf16, tag="sq")
                    nc.vector.tensor_mul(sq[:], xb_sb[:, f, :], xb_sb[:, f, :])
                    nc.tensor.matmul(ps_q[:], onb_sb[:], sq[:],
                                     start=(f == 0), stop=(f == NF - 1))
                mu = tp.tile([1, T], f32, tag="ln_mu")
                msq = tp.tile([1, T], f32, tag="ln_msq")
                mu2 = tp.tile([1, T], f32, tag="ln_mu2")
                var = tp.tile([1, T], f32, tag="ln_var")
                lnv = tp.tile([1, T], f32, tag="ln_lnv")
                ab = tp.tile([1, 2 * T], f32, tag="lnab")
                nc.vector.tensor_scalar_mul(mu[:], ps_s[:], 1.0 / D)
                nc.vector.tensor_scalar_mul(msq[:], ps_q[:], 1.0 / D)
                nc.vector.tensor_mul(mu2[:], mu[:], mu[:])
                # var = (msq + eps) - mu^2  (eps folded in to avoid a const AP)
                nc.vector.scalar_tensor_tensor(var[:], msq[:], 1e-5, mu2[:],
                                               ALU.add, ALU.subtract)
                nc.scalar.activation(lnv[:], var[:], AF.Ln, bias=0.0, scale=1.0)
                nc.scalar.activation(ab[0:1, 0:T], lnv[:], AF.Exp, bias=0.0, scale=-0.5)
                nc.vector.scalar_tensor_tensor(ab[0:1, T:2 * T], mu[:], -1.0,
                                               ab[0:1, 0:T], ALU.mult, ALU.mult)
                ps_bc = sp.tile([128, 2 * T], f32, tag="pbc")
                nc.tensor.matmul(ps_bc[:], onf_sb[:], ab[:], start=True, stop=True)
                for f in range(NF):
                    t = tp.tile([128, T], f32, tag="lntmp")
                    nc.vector.tensor_mul(t[:], x_sb[:, f, :], ps_bc[:, 0:T])
                    nc.vector.tensor_add(n_sb[:, f, :], t[:], ps_bc[:, T:2 * T])

            # ---------------- GEMM helper ----------------
            def emit_gemm(w_d, b_d, rhs_sb, kf_n, of_n, evac, wtag, wpool=wp):
                b_sb = bp.tile([128, of_n], f32, tag="b" + wtag, name="b" + wtag)
                nc.sync.dma_start(b_sb[:], b_d[:])
                w_ap = w_d.ap().rearrange("(kf p) o -> p kf o", p=128)
                order = range(of_n)
                if wtag == "qkv":  # K tiles first, then V, then Q (collective overlap)
                    order = list(range(NF, 2 * NF)) + list(range(2 * NF, 3 * NF)) + list(range(NF))
                for of in order:
                    wt = wpool.tile([128, kf_n, 128], bf16, tag=wtag, name=wtag)
                    nc.sync.dma_start(wt[:], w_ap[:, :, of * 128:(of + 1) * 128])
                    ps = pgp.tile([128, T], f32, tag="pg", name="pg" + wtag)
                    for kf in range(kf_n):
                        nc.tensor.matmul(ps[:], wt[:, kf, :], rhs_sb[:, kf, :],
                                         start=(kf == 0), stop=(kf == kf_n - 1))
                    evac(of, ps, b_sb)

            # ---------------- layers ----------------
            nc.vector.memset(avT_sb[:], 0.0)
            for l in range(n_layers):
                with tc.tile_pool(name=f"lnp{l}a", bufs=1, space="PSUM") as sp:
                    emit_ln(sp)

                def qkv_evac(of, ps, b_sb):
                    dst = (qT_sb, kT_sb, vT_sb)[of // NF]
                    nc.vector.tensor_scalar_add(dst[:, of % NF, :], ps[:],
                                                b_sb[:, of:of + 1])
                emit_gemm(qkvw_d[l], qkvb_d[l], n_sb, NF, 3 * NF, qkv_evac, "qkv")

                # split K to per-head base-0 layout in DRAM, then all-gather
                if do_attn:
                    kin_v = kin_d.ap().rearrange("p (f two t) -> p f two t", f=NF, two=2)
                    nc.sync.dma_start(kin_v[:, :, 0, :], kT_sb[0:64, :, :])
                    nc.sync.dma_start(kin_v[:, :, 1, :], kT_sb[64:128, :, :])
                if do_attn and do_dense:
                    nc.gpsimd.collective_compute("AllGather", ALU.bypass, replica_groups=rg,
                                                 ins=[kin_d[:]], outs=[kout_d[:]])

                # local V transpose -> vpre (token-major, per head), then all-gather
                with tc.tile_pool(name=f"att{l}", bufs=2, space="PSUM") as ap_:
                  if do_attn and attn_stage >= 1:
                    # Q to per-head base-0 (partition shift via SBUF->SBUF DMA)
                    qh_v = qh_sb[:].rearrange("p (f two) t -> p f two t", two=2)
                    nc.sync.dma_start(qh_v[:, :, 0, :], qT_sb[0:64, :, :])
                    nc.sync.dma_start(qh_v[:, :, 1, :], qT_sb[64:128, :, :])
                    # local K per head (round-trip through kin)
                    nc.sync.dma_start(khloc_sb[:],
                                      kin_d.ap().rearrange("p (h t) -> p h t", h=H))
                    for f in range(NF):
                        for c in range(B):
                            ptv = ap_.tile([128, 128], bf16, tag="pt")
                            nc.tensor.transpose(
                                ptv[:], vT_sb[:, f, c * CH:(c + 1) * CH], id_sb[:])
                            nc.vector.tensor_copy(vpre_sb[:, c, 2 * f:2 * f + 2, 0:64],
                                                  ptv[:])
                    if do_dense:
                      nc.sync.dma_start(
                        vin_d.ap().rearrange("p (c h e) -> p c h e", c=B, h=H), vpre_sb[:])
                      nc.gpsimd.collective_compute("AllGather", ALU.bypass, replica_groups=rg,
                                                   ins=[vin_d[:]], outs=[vout_d[:]])

                      # scatter gathered V into per-block SBUF layout
                      for j in range(R):
                          vsrc = vout_d.ap()[j * 128:(j + 1) * 128, :].rearrange(
                              "p (c h e) -> p c h e", c=B, h=H)
                          nc.sync.dma_start(V_sb[:, :, j, :, :], vsrc[:])

                    # attention
                    for c in (range(B) if attn_stage >= 2 else []):
                        Kc = None
                        if do_dense:
                            Kc = kcp.tile([64, R, H, CH], bf16, tag="kc",
                                          name=f"kc{l}{c}")
                            for j in range(R):
                                ksrc = kout_d.ap()[j * 64:(j + 1) * 64, :].rearrange(
                                    "p (h t) -> p h t", h=H)
                                nc.sync.dma_start(Kc[:, j, :, :],
                                                  ksrc[:, :, c * CH:(c + 1) * CH])
                        for hg in range(4):
                            pav = ap_.tile([128, 4, 68], f32, tag="pav")
                            for j in (range(R) if do_dense else []):
                                plg = ap_.tile([128, 4, 128], f32, tag="plg")
                                for h4 in range(4):
                                    h = hg * 4 + h4
                                    nc.tensor.matmul(
                                        plg[:, h4, :],
                                        Kc[:, j, h, :],
                                        qh_sb[:, h, c * CH:(c + 1) * CH],
                                        start=True, stop=True)
                                wgt = tp.tile([128, 4, 128], bf16, tag="wgt")
                                nc.scalar.activation(wgt[:], plg[:], AF.Exp,
                                                     bias=mb_sb[:, j:j + 1], scale=0.125)
                                if attn_stage >= 3:
                                  for h4 in range(4):
                                    h = hg * 4 + h4
                                    # start only on the very first matmul of the
                                    # bank's accumulation group: start=True
                                    # clears has_written for the WHOLE bank.
                                    nc.tensor.matmul(pav[:, h4, 0:65], wgt[:, h4, :],
                                                     V_sb[:, c, j, h, :],
                                                     start=(j == 0 and h4 == 0),
                                                     stop=False, skip_group_check=True)
                            # diagonal block (own K/V, static), triangular mask
                            plgd = ap_.tile([128, 4, 128], f32, tag="plg")
                            for h4 in range(4):
                                h = hg * 4 + h4
                                nc.tensor.matmul(
                                    plgd[:, h4, :],
                                    khloc_sb[:, h, c * CH:(c + 1) * CH],
                                    qh_sb[:, h, c * CH:(c + 1) * CH],
                                    start=True, stop=True)
                            wgtd = tp.tile([128, 4, 128], bf16, tag="wgt")
                            nc.scalar.activation(wgtd[:], plgd[:], AF.Exp,
                                                 bias=0.0, scale=0.125)
                            wgtd2 = tp.tile([128, 4, 128], bf16, tag="wgt")
                            for h4 in range(4):
                                nc.vector.tensor_mul(wgtd2[:, h4, :], wgtd[:, h4, :],
                                                     tri_sb[:])
                            if attn_stage >= 3:
                              for h4 in range(4):
                                h = hg * 4 + h4
                                nc.tensor.matmul(pav[:, h4, 0:65], wgtd2[:, h4, :],
                                                 vpre_sb[:, c, h, :],
                                                 start=(not do_dense and h4 == 0),
                                                 stop=(h4 == 3),
                                                 skip_group_check=True)
                              # normalize by the ones-column row sums
                              rcp = tp.tile([128, 4], f32, tag="rcp")
                              nc.vector.reciprocal(rcp[:], pav[:, :, 64:65])
                              for h4 in range(4):
                                h = hg * 4 + h4
                                nc.vector.tensor_scalar_mul(av_sb[:, c, h, :],
                                                            pav[:, h4, 0:64],
                                                            rcp[:, h4:h4 + 1])
                    # transpose av back to feature-major
                    for c in (range(B) if attn_stage >= 4 else []):
                        for hp in range(NF):
                            pta = ap_.tile([128, 128], bf16, tag="pt")
                            nc.tensor.transpose(pta[:], av_sb[:, c, 2 * hp:2 * hp + 2, :],
                                                id_sb[:])
                            nc.vector.tensor_copy(avT_sb[:, hp, c * CH:(c + 1) * CH],
                                                  pta[:])

                def res_evac(x_slice):
                    def f(of, ps, b_sb):
                        nc.vector.scalar_tensor_tensor(x_sb[:, of, :], ps[:],
                                                       b_sb[:, of:of + 1],
                                                       x_sb[:, of, :],
                                                       ALU.add, ALU.add)
                    return f
                emit_gemm(outw_d[l], outb_d[l], avT_sb, NF, NF, res_evac(None), "outp")

                with tc.tile_pool(name=f"lnp{l}b", bufs=1, space="PSUM") as sp:
                    emit_ln(sp)

                def fc1_evac(of, ps, b_sb):
                    # silu(x+b) = (x+b) * sigmoid(x+b); Silu itself is not in
                    # the CoreSim activation set, so compose it.
                    sg = tp.tile([128, T], bf16, tag="sig")
                    nc.scalar.activation(sg[:], ps[:], AF.Sigmoid,
                                         bias=b_sb[:, of:of + 1], scale=1.0)
                    nc.vector.scalar_tensor_tensor(a_sb[:, of, :], ps[:],
                                                   b_sb[:, of:of + 1], sg[:],
                                                   ALU.add, ALU.mult)
                if do_mlp:
                    emit_gemm(fc1w_d[l], fc1b_d[l], n_sb, NF, 4 * NF, fc1_evac, "fc1")
                    emit_gemm(fc2w_d[l], fc2b_d[l], a_sb, 4 * NF, NF, res_evac(None),
                              "fc2", wpool=wp2)

            # ---------------- final LN + head ----------------
            with tc.tile_pool(name="lnpf", bufs=1, space="PSUM") as sp:
                emit_ln(sp)
            ph = pgp.tile([V, T], f32, tag="pg")
            for f in range(NF):
                nc.tensor.matmul(ph[:], headw_sb[:, f, :], n_sb[:, f, :],
                                 start=(f == 0), stop=(f == NF - 1))
            nc.vector.tensor_scalar_add(out_sb[:], ph[:], headb_sb[:])
            nc.sync.dma_start(out_d[:], out_sb[:])

    nc.compile()
    return nc


def _prep_inputs(params, input_ids):
    """Host-side: fold LN into weights, reorder qkv per-head layout, cast bf16,
    build per-core input maps."""
    f32n = np.float32
    ly = params["layers"]
    tok = np.asarray(params["tok"], f32n)
    pos = np.asarray(params["pos"], f32n)
    ids = np.asarray(input_ids)

    # qkv column permutation: reference layout is per-head [q|k|v] interleave
    perm = np.concatenate([
        np.concatenate([np.arange(HD) + h * 3 * HD + s * HD for h in range(H)])
        for s in range(3)
    ])  # new column n <- old column perm[n]; q block, k block, v block

    in_maps = [dict() for _ in range(R)]
    shared = {}

    def fold(g, b, w, bias):
        w = np.asarray(w, f32n)
        wf = np.asarray(g, f32n)[:, None] * w
        bf = np.asarray(b, f32n) @ w + np.asarray(bias, f32n)
        return wf, bf

    for l in range(L):
        wq, bq = fold(ly["ln1_g"][l], ly["ln1_b"][l], ly["qkv_w"][l], ly["qkv_b"][l])
        wq = wq[:, perm]
        bq = bq[perm]
        w1, b1 = fold(ly["ln2_g"][l], ly["ln2_b"][l], ly["fc1_w"][l], ly["fc1_b"][l])
        shared[f"qkvw{l}"] = wq.astype(ml_dtypes.bfloat16)
        shared[f"qkvb{l}"] = bq.reshape(3 * NF, 128).T.copy().astype(f32n)
        shared[f"outw{l}"] = np.asarray(ly["out_w"][l], f32n).astype(ml_dtypes.bfloat16)
        shared[f"outb{l}"] = np.asarray(ly["out_b"][l], f32n).reshape(NF, 128).T.copy()
        shared[f"fc1w{l}"] = w1.astype(ml_dtypes.bfloat16)
        shared[f"fc1b{l}"] = b1.reshape(4 * NF, 128).T.copy().astype(f32n)
        shared[f"fc2w{l}"] = np.asarray(ly["fc2_w"][l], f32n).astype(ml_dtypes.bfloat16)
        shared[f"fc2b{l}"] = np.asarray(ly["fc2_b"][l], f32n).reshape(NF, 128).T.copy()

    whead, bhead = fold(params["hln_g"], params["hln_b"], params["head_w"],
                        params["head_b"])
    shared["headw"] = whead.astype(ml_dtypes.bfloat16)
    shared["headb"] = bhead.reshape(V, 1).astype(f32n)
    shared["tok"] = tok
    # wgtT layout is [tk, tq]; causal keeps tk <= tq -> upper triangle
    shared["trimask"] = np.triu(np.ones((128, 128), f32n)).astype(ml_dtypes.bfloat16)
    shared["ident"] = np.eye(128, dtype=f32n).astype(ml_dtypes.bfloat16)
    shared["ones_b"] = np.ones((128, 1), f32n).astype(ml_dtypes.bfloat16)
    shared["ones_f"] = np.ones((1, 128), f32n)

    for i in range(R):
        m = in_maps[i]
        m.update(shared)
        ids_i = np.concatenate([ids[b, i * CH:(i + 1) * CH] for b in range(B)])  # [256]
        oh = (ids_i[None, :] == np.arange(V)[:, None]).astype(f32n)  # [4, 256]
        m["onehot"] = oh
        p = pos[i * CH:(i + 1) * CH, :]                       # [128, 1024]
        pT = np.ascontiguousarray(p.T)                        # [1024, 128]
        pT = pT.reshape(NF, 128, CH).transpose(1, 0, 2)       # [128, NF, 128]
        m["posT"] = np.ascontiguousarray(
            np.concatenate([pT, pT], axis=2)).astype(f32n)    # [128, NF, 256]
        mbias = np.zeros((128, 8), f32n)
        mbias[:, i:] = MASK_NEG
        m["maskbias"] = mbias
    return in_maps


def kernel(params, input_ids):
    global _BUILT
    if _BUILT is None:
        _BUILT = _build()
    nc = _BUILT
    in_maps = _prep_inputs(params, input_ids)
    res = run_bass_kernel_spmd(nc, in_maps, core_ids=list(range(R)))
    ids = np.asarray(input_ids)

    logits = np.zeros((B, S, V), np.float32)
    for i in range(R):
        o = res.results[i]["out_logits"]  # [V, 256]
        for b in range(B):
            logits[b, i * CH:(i + 1) * CH, :] = o[:, b * CH:(b + 1) * CH].T
    preds = logits[:, :-1, :]
    labels = ids[:, 1:]
    return preds, labels
